# revision 3
# baseline (speedup 1.0000x reference)
"""LoRA attention kernel for Trainium2, batch-sharded across 8 NeuronCores.

Strategy (v3):
  - Data parallel: batch B=8 -> one batch element per core.
  - LoRA factors and the 1/sqrt(hd) score scale are folded into Wqkv on the
    host (exact algebra, float64).
  - All matmul operands are bf16 (PSUM accumulation stays fp32): halves SBUF
    traffic and enables FWL fast weight loads.
  - q,k are produced transposed ([head_dim, tokens]) directly from x^T so the
    score matmuls need no on-chip transposes. v is produced in natural layout
    with an extra all-ones column per head, so the attention-value matmul
    accumulates the softmax denominators for free in row 64 of its output.
  - Score matmuls run K=64 as row-tiled pairs: even key-tiles on PE rows
    0-63 (tile_position (0,0)), odd key-tiles on rows 64-127 ((64,0)).
    Adjacent issue makes each (even,odd) pair execute concurrently in
    disjoint array row-groups (~310ns/pair vs 430ns serial).
  - Scores land in [128, 1536] PSUM supertiles (3 banks); one ACTIVATE(Exp)
    covers 3 key-tiles, amortizing ScalarE's ~352-cycle per-instruction
    overhead.
  - v-aug construction is interleaved into head 0's first attention chunk
    (it reuses the score supertile banks) so ScalarE starts exp work ~20us
    earlier; output projection for the first 4 token tiles runs during the
    last head's second query chunk.
"""
import numpy as np
import ml_dtypes

import concourse.bass as bass
import concourse.bacc as bacc
import concourse.mybir as mybir
import concourse.tile as tile
from concourse.bass_utils import run_bass_kernel_spmd

F32 = mybir.dt.float32
BF16 = mybir.dt.bfloat16
EXP = mybir.ActivationFunctionType.Exp

B, N, C, H, HD = 8, 1024, 768, 12, 64
CT = C // 128           # 6 contraction tiles over C
QC = N // 512           # 2 query chunks of 512
KT = N // 128           # 8 key tiles of 128
SCALE = HD ** -0.5
N_CORES = 8
VW = (H - 1) * 65 + 128  # vaug tile width (65-pitch heads, widened last read)

_NC_CACHE = None


def _build():
    nc = bacc.Bacc(None, target_bir_lowering=False)

    xT = nc.dram_tensor("xT", [C, N], BF16, kind="ExternalInput")
    wqk = nc.dram_tensor("wqk", [H, 128, C], BF16, kind="ExternalInput")
    wv = nc.dram_tensor("wv", [CT, 128, C], BF16, kind="ExternalInput")
    wpt = nc.dram_tensor("wpt", [CT, 128, C], BF16, kind="ExternalInput")
    bias = nc.dram_tensor("bias", [1, C], F32, kind="ExternalInput")
    y = nc.dram_tensor("y", [N, C], F32, kind="ExternalOutput")

    from contextlib import ExitStack
    with tile.TileContext(nc) as tc:
        with ExitStack() as ctx:
            pool = lambda name, bufs, **kw: ctx.enter_context(
                tc.tile_pool(name=name, bufs=bufs, **kw))
            xt_pool = pool("xt", CT)
            wqk_pool = pool("wqkp", 3)
            w768_pool = pool("w768", 2 * CT)      # wv + wpt
            vaug_pool = pool("vaug", KT)
            st_pool = pool("stp", 6)
            ktq_pool = pool("ktq", 8)
            et_pool = pool("etp", 6)
            avs_pool = pool("avsp", 3)
            iv_pool = pool("ivp", 3)
            bc_pool = pool("bcp", 3)
            ost_pool = pool("ostp", 3)
            out_pool = pool("outp", CT)
            y_pool = pool("yp", 3)
            cst_pool = pool("cst", 1)
            sc_ps = pool("sc_ps", 2, space="PSUM")    # [128,1536] supertiles
            av_ps = pool("av_ps", 1, space="PSUM")
            qk_ps = pool("qk_ps", 1, space="PSUM")

            # ---- PE warm-up: bridge the DMA lead-in so the HAM clock gate
            # opens before real work arrives ---------------------------------
            wur = cst_pool.tile([128, 512], BF16, tag="wur")
            nc.vector.memset(wur, 0.0)

            def warmup(n, label):
                for i in range(n):
                    wps = sc_ps.tile([128, 1536], F32, tag="sc",
                                     name=f"wu{label}_{i}")
                    nc.tensor.matmul(wps[:, 0:512], wur[:, 0:128], wur,
                                     start=True, stop=True)

            warmup(12, "a")

            # ---- loads -----------------------------------------------------
            def load_wqk(h):
                wt = wqk_pool.tile([128, C], BF16, tag="wqk", name=f"wqk{h}")
                nc.sync.dma_start(out=wt, in_=wqk[h, :, :])
                return wt

            wts0 = load_wqk(0)

            xt = []
            for c in range(CT):
                t = xt_pool.tile([128, N], BF16, tag="xt", name=f"xt{c}")
                nc.sync.dma_start(out=t, in_=xT[c * 128:(c + 1) * 128, :])
                xt.append(t)

            bias_bc = cst_pool.tile([128, C], F32, tag="biasbc")
            nc.sync.dma_start(out=bias_bc, in_=bias[:, :].to_broadcast([128, C]))
            ones12 = cst_pool.tile([128, H], BF16, tag="ones12")
            nc.vector.memset(ones12, 1.0)

            wvt = []
            for c in range(CT):
                t = w768_pool.tile([128, C], BF16, tag="w768", name=f"wv{c}")
                nc.sync.dma_start(out=t, in_=wv[c, :, :])
                wvt.append(t)

            # ---- per-head q/k projection -----------------------------------
            def qk_project(h, wt):
                """q (rows 0-63) and k (rows 64-127), transposed layout.
                Returns per-qc lists (st, kt_t, qdup)."""
                sts, kts, qds = [], [], []
                for qc in range(QC):
                    pqk = qk_ps.tile([128, 512], F32, tag="qk",
                                     name=f"pqk{h}_{qc}")
                    for c in range(CT):
                        nc.tensor.matmul(
                            pqk, wt[:, c * 128:(c + 1) * 128],
                            xt[c][:, qc * 512:(qc + 1) * 512],
                            start=(c == 0), stop=(c == CT - 1),
                        )
                    st = st_pool.tile([128, 512], BF16, tag="st",
                                      name=f"st{h}_{qc}")
                    nc.vector.tensor_copy(st, pqk)
                    # k rows of EVEN key-chunks also needed at partitions 0-63
                    # (row-tile 0); q rows duplicated at partitions 64-127 for
                    # the odd-chunk matmuls on row-tile 1.
                    kt_t = ktq_pool.tile([128, 512], BF16, tag="ktq",
                                         name=f"kt{h}_{qc}")
                    nc.sync.dma_start(out=kt_t[0:64, :], in_=st[64:128, :])
                    qd = ktq_pool.tile([128, 512], BF16, tag="ktq",
                                       name=f"qd{h}_{qc}")
                    nc.sync.dma_start(out=qd[64:128, :], in_=st[0:64, :])
                    sts.append(st)
                    kts.append(kt_t)
                    qds.append(qd)
                return sts, kts, qds

            # ---- v_aug[tt] builder (interleaved into head 0's attention) ---
            vaug = [None] * KT

            def make_vaug(tt):
                def go():
                    pv = sc_ps.tile([128, 1536], F32, tag="sc",
                                    name=f"pv{tt}")
                    for c in range(CT):
                        xs = xt[c][:, tt * 128:(tt + 1) * 128]
                        nc.tensor.matmul(pv[:, 0:512], xs, wvt[c][:, 0:512],
                                         start=(c == 0), stop=(c == CT - 1))
                        nc.tensor.matmul(pv[:, 512:768], xs,
                                         wvt[c][:, 512:768],
                                         start=(c == 0), stop=(c == CT - 1))
                    va = vaug_pool.tile([128, VW], BF16, tag="vaug",
                                        name=f"vaug{tt}")
                    dst = bass.AP(tensor=va.tensor, offset=va.offset,
                                  ap=[va.ap[0], [65, H], [1, 64]])
                    nc.vector.tensor_copy(dst, pv[:, 0:768])
                    ones_ap = bass.AP(tensor=va.tensor, offset=va.offset + 64,
                                      ap=[va.ap[0], [65, H]])
                    nc.vector.tensor_copy(ones_ap, ones12)
                    vaug[tt] = va
                return go

            # ---- output accumulator tiles (c-major, [128, N]) --------------
            outT = [out_pool.tile([128, N], BF16, tag="outT", name=f"outT{i}")
                    for i in range(CT)]

            def emit_sc(slot, h, qc, kt, sts, kts, qds):
                cs = slice((kt % 4) * 128, (kt % 4 + 1) * 128)
                if kt % 2 == 0:
                    nc.tensor.matmul(slot, kts[kt // 4][0:64, cs],
                                     sts[qc][0:64, :], start=True, stop=True,
                                     tile_position=(0, 0))
                else:
                    nc.tensor.matmul(slot, sts[kt // 4][64:128, cs],
                                     qds[qc][64:128, :], start=True, stop=True,
                                     tile_position=(64, 0))

            def attn_chunk(h, qc, sts, kts, qds, fillers=None):
                """One (head, query-chunk): scores, exp, attn*v, softmax norm.
                fillers: 3 optional callables run between av groups (PE work
                that overlaps the exp pipeline)."""
                f0, f1, f2 = fillers if fillers else (None, None, None)
                av = av_ps.tile([128, 512], F32, tag="av", name=f"av{h}_{qc}")
                psA = sc_ps.tile([128, 1536], F32, tag="sc",
                                 name=f"scA{h}_{qc}")
                psB = sc_ps.tile([128, 1536], F32, tag="sc",
                                 name=f"scB{h}_{qc}")
                # six score matmuls as three adjacent (even,odd) pairs
                for kt in range(6):
                    ps, i = (psA, kt) if kt < 3 else (psB, kt - 3)
                    emit_sc(ps[:, i * 512:(i + 1) * 512], h, qc, kt,
                            sts, kts, qds)
                etA = et_pool.tile([128, 1536], BF16, tag="et",
                                   name=f"etA{h}_{qc}")
                nc.scalar.activation(out=etA, in_=psA, func=EXP)
                etB = et_pool.tile([128, 1536], BF16, tag="et",
                                   name=f"etB{h}_{qc}")
                nc.scalar.activation(out=etB, in_=psB, func=EXP)
                if f0:
                    f0()
                for kt in range(3):
                    nc.tensor.matmul(av, vaug[kt][:, h * 65:h * 65 + 128],
                                     etA[:, kt * 512:(kt + 1) * 512],
                                     start=(kt == 0), stop=False)
                if f1:
                    f1()
                for kt in range(3, 6):
                    nc.tensor.matmul(av, vaug[kt][:, h * 65:h * 65 + 128],
                                     etB[:, (kt - 3) * 512:(kt - 2) * 512],
                                     start=False, stop=False)
                psC = sc_ps.tile([128, 1536], F32, tag="sc",
                                 name=f"scC{h}_{qc}")
                emit_sc(psC[:, 0:512], h, qc, 6, sts, kts, qds)
                emit_sc(psC[:, 512:1024], h, qc, 7, sts, kts, qds)
                etC = et_pool.tile([128, 1536], BF16, tag="et",
                                   name=f"etC{h}_{qc}")
                nc.scalar.activation(out=etC[:, 0:1024], in_=psC[:, 0:1024],
                                     func=EXP)
                if f2:
                    f2()
                for kt in range(6, 8):
                    nc.tensor.matmul(av, vaug[kt][:, h * 65:h * 65 + 128],
                                     etC[:, (kt - 6) * 512:(kt - 5) * 512],
                                     start=False, stop=(kt == KT - 1))
                # drain the psum quickly to release the bank; normalization
                # then runs off the PE critical path
                avs = avs_pool.tile([65, 512], F32, tag="avs",
                                    name=f"avs{h}_{qc}")
                nc.vector.tensor_copy(avs, av[0:65, :])
                # row 64 of avs = softmax denominators for this q chunk.
                sm0 = iv_pool.tile([1, 512], F32, tag="sm0",
                                   name=f"sm0{h}_{qc}")
                nc.sync.dma_start(out=sm0, in_=avs[64:65, :])
                iv0 = iv_pool.tile([1, 512], F32, tag="iv0",
                                   name=f"iv0{h}_{qc}")
                nc.vector.reciprocal_approx_fast(out=iv0, in_=sm0)
                bc = bc_pool.tile([64, 512], F32, tag="bc", name=f"bc{h}_{qc}")
                nc.gpsimd.partition_broadcast(bc, iv0)
                ct_i = h // 2
                if h % 2 == 0:
                    nc.vector.tensor_mul(
                        outT[ct_i][0:64, qc * 512:(qc + 1) * 512],
                        avs[0:64, :], bc)
                else:
                    ost = ost_pool.tile([64, 512], BF16, tag="ost",
                                        name=f"ost{h}_{qc}")
                    nc.vector.tensor_mul(ost, avs[0:64, :], bc)
                    nc.sync.dma_start(
                        out=outT[ct_i][64:128, qc * 512:(qc + 1) * 512],
                        in_=ost)

            wptt = None

            def emit_proj(tts):
                for tt in tts:
                    py = sc_ps.tile([128, 1536], F32, tag="sc",
                                    name=f"py{tt}")
                    for c in range(CT):
                        os_ = outT[c][:, tt * 128:(tt + 1) * 128]
                        nc.tensor.matmul(py[:, 0:512], os_, wptt[c][:, 0:512],
                                         start=(c == 0), stop=(c == CT - 1))
                        nc.tensor.matmul(py[:, 512:768], os_,
                                         wptt[c][:, 512:768],
                                         start=(c == 0), stop=(c == CT - 1))
                    ysb = y_pool.tile([128, C], F32, tag="y", name=f"y{tt}")
                    nc.vector.tensor_add(ysb, py[:, 0:768], bias_bc)
                    nc.sync.dma_start(out=y[tt * 128:(tt + 1) * 128, :],
                                      in_=ysb)

            # ---- head loop -------------------------------------------------
            head_order = list(range(H))
            head_order[10], head_order[11] = head_order[11], head_order[10]
            vg = [make_vaug(tt) for tt in range(KT)]

            def run3(a, b, c=None):
                def go():
                    a()
                    b()
                    if c:
                        c()
                return go

            last = head_order[-1]
            for h in head_order:
                sts, kts, qds = qk_project(h, wts0 if h == 0 else load_wqk(h))
                if h == 0:
                    attn_chunk(h, 0, sts, kts, qds,
                               fillers=(run3(vg[0], vg[1], vg[2]),
                                        run3(vg[3], vg[4], vg[5]),
                                        run3(vg[6], vg[7])))
                    # NOTE: f0 runs before the first av group, so vaug[0..2]
                    # exist in time; f1 before group B avs, f2 before group C.
                    attn_chunk(h, 1, sts, kts, qds)
                else:
                    attn_chunk(h, 0, sts, kts, qds)
                    if h == last:
                        emit_proj(range(0, 4))
                    attn_chunk(h, 1, sts, kts, qds)

                if h == 5:
                    # prefetch output-projection weights mid-flight
                    wptt = []
                    for c in range(CT):
                        t = w768_pool.tile([128, C], BF16, tag="w768",
                                           name=f"wpt{c}")
                        nc.sync.dma_start(out=t, in_=wpt[c, :, :])
                        wptt.append(t)

            emit_proj(range(4, KT))

    nc.finalize()
    return nc


def _get_nc():
    global _NC_CACHE
    if _NC_CACHE is None:
        _NC_CACHE = _build()
    return _NC_CACHE


def _host_prep(x, Wqkv, Wproj, bproj, Aq, Bq, Av, Bv):
    """Fold LoRA + score scale into the weights; lay out and cast to bf16."""
    W = Wqkv.astype(np.float64)
    Wq = W[0:C].reshape(H, HD, C)
    Wk = W[C:2 * C].reshape(H, HD, C)
    Wv_ = W[2 * C:3 * C].reshape(H, HD, C)
    ABq = Aq.astype(np.float64) @ Bq.astype(np.float64)   # [HD, HD]
    ABv = Av.astype(np.float64) @ Bv.astype(np.float64)
    Wq = Wq + np.einsum('ed,hec->hdc', ABq, Wq)           # (I+AB).T @ Wq per head
    Wv_ = Wv_ + np.einsum('ed,hec->hdc', ABv, Wv_)
    Wq = Wq * SCALE                                       # fold softmax scale

    # wqk[h] = [K=c-rows(128), 6 c-tiles of (q_h cols(64) ++ k_h cols(64))]
    wqk = np.empty((H, 128, C), np.float32)
    for h in range(H):
        for c in range(CT):
            cs = slice(c * 128, (c + 1) * 128)
            wqk[h, :, c * 128:c * 128 + 64] = Wq[h][:, cs].T
            wqk[h, :, c * 128 + 64:(c + 1) * 128] = Wk[h][:, cs].T

    # wv[c] = [K=c-rows(128), all 768 v output features]
    WvT = Wv_.reshape(C, C).T.astype(np.float32)          # [c_in, v_out]
    wv = np.ascontiguousarray(WvT.reshape(CT, 128, C))

    # wpt[c] = Wproj.T c-tiles: [K=c(128), e(768)]
    WpT = Wproj.astype(np.float32).T                      # [c, e]
    wpt = np.ascontiguousarray(WpT.reshape(CT, 128, C))

    bf = ml_dtypes.bfloat16
    wqk = wqk.astype(bf)
    wv = wv.astype(bf)
    wpt = wpt.astype(bf)
    bias = bproj.astype(np.float32).reshape(1, C)

    per_core = []
    for b in range(B):
        xTb = np.ascontiguousarray(x[b].astype(np.float32).T.astype(bf))
        per_core.append({"xT": xTb, "wqk": wqk, "wv": wv, "wpt": wpt,
                         "bias": bias})
    return per_core


def kernel(x, Wqkv, Wproj, bproj, Aq, Bq, Av, Bv, _trace=False):
    x = np.asarray(x)
    in_maps = _host_prep(np.asarray(x), np.asarray(Wqkv), np.asarray(Wproj),
                         np.asarray(bproj), np.asarray(Aq), np.asarray(Bq),
                         np.asarray(Av), np.asarray(Bv))
    nc = _get_nc()
    res = run_bass_kernel_spmd(nc, in_maps, core_ids=list(range(N_CORES)),
                               trace=_trace)
    out = np.stack([res.results[b]["y"] for b in range(B)], axis=0)
    if _trace:
        kernel._last_result = res
    return out.astype(np.float32)


# revision 4
# speedup vs baseline: 1.0714x; 1.0714x over previous
"""LoRA attention kernel for Trainium2, batch-sharded across 8 NeuronCores.

Strategy (v4):
  - Data parallel: batch B=8 -> one batch element per core.
  - LoRA factors and the 1/sqrt(hd) score scale are folded into Wqkv on the
    host (exact algebra, float64).
  - All matmul operands are bf16 (PSUM accumulation stays fp32): halves SBUF
    traffic and enables FWL fast weight loads.
  - q,k are produced transposed ([head_dim, tokens]) directly from x^T so the
    score matmuls need no on-chip transposes. v is produced in natural layout
    with an extra all-ones column per head, so the attention-value matmul
    accumulates the softmax denominators for free in row 64 of its output.
  - Score matmuls run K=64 as row-tiled pairs: even key-tiles on PE rows
    0-63 (tile_position (0,0)), odd key-tiles on rows 64-127 ((64,0)).
    Adjacent issue makes each (even,odd) pair execute concurrently in
    disjoint array row-groups (~310ns/pair vs 430ns serial).
  - Scores land in [128, 1024] PSUM supertiles (2 banks, 3 rotating bufs);
    one ACTIVATE(Exp) covers 2 key-tiles, amortizing ScalarE's ~352-cycle
    per-instruction overhead. ScalarE is the pacing engine: the whole kernel
    is software-pipelined so exp inputs are always ready ahead of it --
    score groups of chunk s are emitted while attn*v groups run 1-3 chunks
    behind (v-aug construction fills the early-chunk PE slack).
"""
import numpy as np
import ml_dtypes

import concourse.bass as bass
import concourse.bacc as bacc
import concourse.mybir as mybir
import concourse.tile as tile
from concourse.bass_utils import run_bass_kernel_spmd

F32 = mybir.dt.float32
BF16 = mybir.dt.bfloat16
EXP = mybir.ActivationFunctionType.Exp

B, N, C, H, HD = 8, 1024, 768, 12, 64
CT = C // 128           # 6 contraction tiles over C
QC = N // 512           # 2 query chunks of 512
KT = N // 128           # 8 key tiles of 128
SCALE = HD ** -0.5
N_CORES = 8
VW = (H - 1) * 65 + 128  # vaug tile width (65-pitch heads, widened last read)

_NC_CACHE = None


def _build():
    nc = bacc.Bacc(None, target_bir_lowering=False)

    xT = nc.dram_tensor("xT", [C, N], BF16, kind="ExternalInput")
    wqk = nc.dram_tensor("wqk", [H, 128, C], BF16, kind="ExternalInput")
    wv = nc.dram_tensor("wv", [CT, 128, C], BF16, kind="ExternalInput")
    wpt = nc.dram_tensor("wpt", [CT, 128, C], BF16, kind="ExternalInput")
    bias = nc.dram_tensor("bias", [1, C], F32, kind="ExternalInput")
    y = nc.dram_tensor("y", [N, C], F32, kind="ExternalOutput")

    from contextlib import ExitStack
    with tile.TileContext(nc) as tc:
        with ExitStack() as ctx:
            pool = lambda name, bufs, **kw: ctx.enter_context(
                tc.tile_pool(name=name, bufs=bufs, **kw))
            xt_pool = pool("xt", CT)
            wqk_pool = pool("wqkp", 3)
            w768_pool = pool("w768", 2 * CT)      # wv + wpt
            vaug_pool = pool("vaug", KT)
            st_pool = pool("stp", 6)
            ktq_pool = pool("ktq", 8)
            et_pool = pool("etp", 16)
            avs_pool = pool("avsp", 3)
            iv_pool = pool("ivp", 4)
            bc_pool = pool("bcp", 3)
            ost_pool = pool("ostp", 3)
            out_pool = pool("outp", CT)
            y_pool = pool("yp", 3)
            cst_pool = pool("cst", 1)
            sc_ps = pool("sc_ps", 3, space="PSUM")    # [128,1024] supertiles
            av_ps = pool("av_ps", 1, space="PSUM")
            qk_ps = pool("qk_ps", 1, space="PSUM")

            # ---- PE warm-up: bridge the DMA lead-in so the HAM clock gate
            # opens before real work arrives ---------------------------------
            wur = cst_pool.tile([128, 512], BF16, tag="wur")
            nc.vector.memset(wur, 0.0)
            for i in range(14):
                wps = qk_ps.tile([128, 512], F32, tag="qk", name=f"wu{i}")
                nc.tensor.matmul(wps, wur[:, 0:128], wur,
                                 start=True, stop=True)

            # ---- loads -----------------------------------------------------
            def load_wqk(h):
                wt = wqk_pool.tile([128, C], BF16, tag="wqk", name=f"wqk{h}")
                nc.sync.dma_start(out=wt, in_=wqk[h, :, :])
                return wt

            wts0 = load_wqk(0)

            xt = []
            for c in range(CT):
                t = xt_pool.tile([128, N], BF16, tag="xt", name=f"xt{c}")
                nc.sync.dma_start(out=t, in_=xT[c * 128:(c + 1) * 128, :])
                xt.append(t)

            bias_bc = cst_pool.tile([128, C], F32, tag="biasbc")
            nc.sync.dma_start(out=bias_bc, in_=bias[:, :].to_broadcast([128, C]))
            ones12 = cst_pool.tile([128, H], BF16, tag="ones12")
            nc.vector.memset(ones12, 1.0)

            wvt = []
            for c in range(CT):
                t = w768_pool.tile([128, C], BF16, tag="w768", name=f"wv{c}")
                nc.sync.dma_start(out=t, in_=wv[c, :, :])
                wvt.append(t)

            # ---- per-head q/k projection -----------------------------------
            def qk_project(h, wt):
                """q (rows 0-63) and k (rows 64-127), transposed layout."""
                sts, kts, qds = [], [], []
                for qc in range(QC):
                    pqk = qk_ps.tile([128, 512], F32, tag="qk",
                                     name=f"pqk{h}_{qc}")
                    for c in range(CT):
                        nc.tensor.matmul(
                            pqk, wt[:, c * 128:(c + 1) * 128],
                            xt[c][:, qc * 512:(qc + 1) * 512],
                            start=(c == 0), stop=(c == CT - 1),
                        )
                    st = st_pool.tile([128, 512], BF16, tag="st",
                                      name=f"st{h}_{qc}")
                    nc.vector.tensor_copy(st, pqk)
                    # k rows of EVEN key-chunks also needed at partitions 0-63
                    # (row-tile 0); q rows duplicated at partitions 64-127 for
                    # the odd-chunk matmuls on row-tile 1.
                    kt_t = ktq_pool.tile([128, 512], BF16, tag="ktq",
                                         name=f"kt{h}_{qc}")
                    nc.sync.dma_start(out=kt_t[0:64, :], in_=st[64:128, :])
                    qd = ktq_pool.tile([128, 512], BF16, tag="ktq",
                                       name=f"qd{h}_{qc}")
                    nc.sync.dma_start(out=qd[64:128, :], in_=st[0:64, :])
                    sts.append(st)
                    kts.append(kt_t)
                    qds.append(qd)
                return sts, kts, qds

            # ---- v_aug[tt] builders ---------------------------------------
            vaug = [None] * KT

            def build_vaug(tt):
                pv = sc_ps.tile([128, 1024], F32, tag="sc", name=f"pv{tt}")
                for c in range(CT):
                    xs = xt[c][:, tt * 128:(tt + 1) * 128]
                    nc.tensor.matmul(pv[:, 0:512], xs, wvt[c][:, 0:512],
                                     start=(c == 0), stop=(c == CT - 1))
                    nc.tensor.matmul(pv[:, 512:768], xs, wvt[c][:, 512:768],
                                     start=(c == 0), stop=(c == CT - 1))
                va = vaug_pool.tile([128, VW], BF16, tag="vaug",
                                    name=f"vaug{tt}")
                dst = bass.AP(tensor=va.tensor, offset=va.offset,
                              ap=[va.ap[0], [65, H], [1, 64]])
                nc.vector.tensor_copy(dst, pv[:, 0:768])
                ones_ap = bass.AP(tensor=va.tensor, offset=va.offset + 64,
                                  ap=[va.ap[0], [65, H]])
                nc.vector.tensor_copy(ones_ap, ones12)
                vaug[tt] = va

            # ---- output accumulator tiles (c-major, [128, N]) --------------
            outT = [out_pool.tile([128, N], BF16, tag="outT", name=f"outT{i}")
                    for i in range(CT)]

            def emit_sc(slot, qc, kt, sk):
                sts, kts, qds = sk
                cs = slice((kt % 4) * 128, (kt % 4 + 1) * 128)
                if kt % 2 == 0:
                    nc.tensor.matmul(slot, kts[kt // 4][0:64, cs],
                                     sts[qc][0:64, :], start=True, stop=True,
                                     tile_position=(0, 0))
                else:
                    nc.tensor.matmul(slot, sts[kt // 4][64:128, cs],
                                     qds[qc][64:128, :], start=True, stop=True,
                                     tile_position=(64, 0))

            def emit_sc_chunk(h, qc, sk):
                """Scores + exp for one (head, query-chunk): 4 row-tiled
                pairs into [128,1024] supertiles, one Exp each."""
                ets = []
                for g in range(4):
                    ps = sc_ps.tile([128, 1024], F32, tag="sc",
                                    name=f"sc{h}_{qc}_{g}")
                    emit_sc(ps[:, 0:512], qc, 2 * g, sk)
                    emit_sc(ps[:, 512:1024], qc, 2 * g + 1, sk)
                    et = et_pool.tile([128, 1024], BF16, tag="et",
                                      name=f"et{h}_{qc}_{g}")
                    nc.scalar.activation(out=et, in_=ps, func=EXP)
                    ets.append(et)
                return ets

            def emit_av_chunk(h, qc, ets):
                """attn*v accumulation + softmax normalization for a chunk."""
                av = av_ps.tile([128, 512], F32, tag="av", name=f"av{h}_{qc}")
                for kt in range(KT):
                    nc.tensor.matmul(av, vaug[kt][:, h * 65:h * 65 + 128],
                                     ets[kt // 2][:, (kt % 2) * 512:
                                                  (kt % 2 + 1) * 512],
                                     start=(kt == 0), stop=(kt == KT - 1))
                avs = avs_pool.tile([65, 512], F32, tag="avs",
                                    name=f"avs{h}_{qc}")
                nc.vector.tensor_copy(avs, av[0:65, :])
                # row 64 of avs = softmax denominators for this q chunk.
                sm0 = iv_pool.tile([1, 512], F32, tag="sm0",
                                   name=f"sm0{h}_{qc}")
                nc.sync.dma_start(out=sm0, in_=avs[64:65, :])
                iv0 = iv_pool.tile([1, 512], F32, tag="iv0",
                                   name=f"iv0{h}_{qc}")
                nc.vector.reciprocal_approx_fast(out=iv0, in_=sm0)
                bc = bc_pool.tile([64, 512], F32, tag="bc", name=f"bc{h}_{qc}")
                nc.gpsimd.partition_broadcast(bc, iv0)
                ct_i = h // 2
                if h % 2 == 0:
                    nc.vector.tensor_mul(
                        outT[ct_i][0:64, qc * 512:(qc + 1) * 512],
                        avs[0:64, :], bc)
                else:
                    ost = ost_pool.tile([64, 512], BF16, tag="ost",
                                        name=f"ost{h}_{qc}")
                    nc.vector.tensor_mul(ost, avs[0:64, :], bc)
                    nc.sync.dma_start(
                        out=outT[ct_i][64:128, qc * 512:(qc + 1) * 512],
                        in_=ost)

            wptt = None

            def emit_proj(tts):
                for tt in tts:
                    py = sc_ps.tile([128, 1024], F32, tag="sc",
                                    name=f"py{tt}")
                    for c in range(CT):
                        os_ = outT[c][:, tt * 128:(tt + 1) * 128]
                        nc.tensor.matmul(py[:, 0:512], os_, wptt[c][:, 0:512],
                                         start=(c == 0), stop=(c == CT - 1))
                        nc.tensor.matmul(py[:, 512:768], os_,
                                         wptt[c][:, 512:768],
                                         start=(c == 0), stop=(c == CT - 1))
                    ysb = y_pool.tile([128, C], F32, tag="y", name=f"y{tt}")
                    nc.vector.tensor_add(ysb, py[:, 0:768], bias_bc)
                    nc.sync.dma_start(out=y[tt * 128:(tt + 1) * 128, :],
                                      in_=ysb)

            # ---- software-pipelined schedule -------------------------------
            head_order = list(range(H))
            head_order[10], head_order[11] = head_order[11], head_order[10]
            chunks = [(h, qc) for h in head_order for qc in range(QC)]
            pv_slots = {0: (0, 1), 1: (2, 3), 2: (4, 5), 3: (6, 7)}

            sk_by_head = {head_order[0]: qk_project(head_order[0], wts0)}
            ets_store = {}
            av_done = 0

            for s, (h, qc) in enumerate(chunks):
                ets_store[s] = emit_sc_chunk(h, qc, sk_by_head[h])
                if qc == 0 and 2 * (s // 2) + 2 < len(chunks):
                    hn = chunks[s + 2][0]
                    sk_by_head[hn] = qk_project(hn, load_wqk(hn))
                if s in pv_slots:
                    for tt in pv_slots[s]:
                        build_vaug(tt)
                if (h, qc) == (5, 1):
                    wptt = []
                    for c in range(CT):
                        t = w768_pool.tile([128, C], BF16, tag="w768",
                                           name=f"wpt{c}")
                        nc.sync.dma_start(out=t, in_=wpt[c, :, :])
                        wptt.append(t)
                target = 0 if s < 3 else (s - 2 if s < 8 else s)
                while av_done < target:
                    ch, cq = chunks[av_done]
                    emit_av_chunk(ch, cq, ets_store.pop(av_done))
                    av_done += 1
                    if av_done == len(chunks) - 1:
                        emit_proj(range(0, 4))

            while av_done < len(chunks):
                ch, cq = chunks[av_done]
                emit_av_chunk(ch, cq, ets_store.pop(av_done))
                av_done += 1
                if av_done == len(chunks) - 1:
                    emit_proj(range(0, 4))
            emit_proj(range(4, KT))

    nc.finalize()
    return nc


def _get_nc():
    global _NC_CACHE
    if _NC_CACHE is None:
        _NC_CACHE = _build()
    return _NC_CACHE


def _host_prep(x, Wqkv, Wproj, bproj, Aq, Bq, Av, Bv):
    """Fold LoRA + score scale into the weights; lay out and cast to bf16."""
    W = Wqkv.astype(np.float64)
    Wq = W[0:C].reshape(H, HD, C)
    Wk = W[C:2 * C].reshape(H, HD, C)
    Wv_ = W[2 * C:3 * C].reshape(H, HD, C)
    ABq = Aq.astype(np.float64) @ Bq.astype(np.float64)   # [HD, HD]
    ABv = Av.astype(np.float64) @ Bv.astype(np.float64)
    Wq = Wq + np.einsum('ed,hec->hdc', ABq, Wq)           # (I+AB).T @ Wq per head
    Wv_ = Wv_ + np.einsum('ed,hec->hdc', ABv, Wv_)
    Wq = Wq * SCALE                                       # fold softmax scale

    # wqk[h] = [K=c-rows(128), 6 c-tiles of (q_h cols(64) ++ k_h cols(64))]
    wqk = np.empty((H, 128, C), np.float32)
    for h in range(H):
        for c in range(CT):
            cs = slice(c * 128, (c + 1) * 128)
            wqk[h, :, c * 128:c * 128 + 64] = Wq[h][:, cs].T
            wqk[h, :, c * 128 + 64:(c + 1) * 128] = Wk[h][:, cs].T

    # wv[c] = [K=c-rows(128), all 768 v output features]
    WvT = Wv_.reshape(C, C).T.astype(np.float32)          # [c_in, v_out]
    wv = np.ascontiguousarray(WvT.reshape(CT, 128, C))

    # wpt[c] = Wproj.T c-tiles: [K=c(128), e(768)]
    WpT = Wproj.astype(np.float32).T                      # [c, e]
    wpt = np.ascontiguousarray(WpT.reshape(CT, 128, C))

    bf = ml_dtypes.bfloat16
    wqk = wqk.astype(bf)
    wv = wv.astype(bf)
    wpt = wpt.astype(bf)
    bias = bproj.astype(np.float32).reshape(1, C)

    per_core = []
    for b in range(B):
        xTb = np.ascontiguousarray(x[b].astype(np.float32).T.astype(bf))
        per_core.append({"xT": xTb, "wqk": wqk, "wv": wv, "wpt": wpt,
                         "bias": bias})
    return per_core


def kernel(x, Wqkv, Wproj, bproj, Aq, Bq, Av, Bv, _trace=False):
    x = np.asarray(x)
    in_maps = _host_prep(np.asarray(x), np.asarray(Wqkv), np.asarray(Wproj),
                         np.asarray(bproj), np.asarray(Aq), np.asarray(Bq),
                         np.asarray(Av), np.asarray(Bv))
    nc = _get_nc()
    res = run_bass_kernel_spmd(nc, in_maps, core_ids=list(range(N_CORES)),
                               trace=_trace)
    out = np.stack([res.results[b]["y"] for b in range(B)], axis=0)
    if _trace:
        kernel._last_result = res
    return out.astype(np.float32)


# revision 8
# speedup vs baseline: 1.1239x; 1.0490x over previous
"""LoRA attention kernel for Trainium2, batch-sharded across 8 NeuronCores.

Strategy (v4):
  - Data parallel: batch B=8 -> one batch element per core.
  - LoRA factors and the 1/sqrt(hd) score scale are folded into Wqkv on the
    host (exact algebra, float64).
  - All matmul operands are bf16 (PSUM accumulation stays fp32): halves SBUF
    traffic and enables FWL fast weight loads.
  - q,k are produced transposed ([head_dim, tokens]) directly from x^T so the
    score matmuls need no on-chip transposes. v is produced in natural layout
    with an extra all-ones column per head, so the attention-value matmul
    accumulates the softmax denominators for free in row 64 of its output.
  - Score matmuls run K=64 as row-tiled pairs: even key-tiles on PE rows
    0-63 (tile_position (0,0)), odd key-tiles on rows 64-127 ((64,0)).
    Adjacent issue makes each (even,odd) pair execute concurrently in
    disjoint array row-groups (~310ns/pair vs 430ns serial).
  - Scores land in [128, 1024] PSUM supertiles (2 banks, 3 rotating bufs);
    one ACTIVATE(Exp) covers 2 key-tiles, amortizing ScalarE's ~352-cycle
    per-instruction overhead. ScalarE is the pacing engine: the whole kernel
    is software-pipelined so exp inputs are always ready ahead of it --
    score groups of chunk s are emitted while attn*v groups run 1-3 chunks
    behind (v-aug construction fills the early-chunk PE slack).
"""
import numpy as np
import ml_dtypes

import concourse.bass as bass
import concourse.bacc as bacc
import concourse.mybir as mybir
import concourse.tile as tile
from concourse.bass_utils import run_bass_kernel_spmd

F32 = mybir.dt.float32
BF16 = mybir.dt.bfloat16
EXP = mybir.ActivationFunctionType.Exp

B, N, C, H, HD = 8, 1024, 768, 12, 64
CT = C // 128           # 6 contraction tiles over C
QC = N // 512           # 2 query chunks of 512
KT = N // 128           # 8 key tiles of 128
SCALE = HD ** -0.5
N_CORES = 8
VW = (H - 1) * 65 + 128  # vaug tile width (65-pitch heads, widened last read)

_NC_CACHE = None


def _build():
    nc = bacc.Bacc(None, target_bir_lowering=False)

    xT = nc.dram_tensor("xT", [C, N], BF16, kind="ExternalInput")
    wqk = nc.dram_tensor("wqk", [H, 128, C], BF16, kind="ExternalInput")
    wv = nc.dram_tensor("wv", [CT, 128, C], BF16, kind="ExternalInput")
    wpt = nc.dram_tensor("wpt", [CT, 128, C], BF16, kind="ExternalInput")
    bias = nc.dram_tensor("bias", [1, C], F32, kind="ExternalInput")
    y = nc.dram_tensor("y", [N, C], F32, kind="ExternalOutput")

    from contextlib import ExitStack
    with tile.TileContext(nc) as tc:
        with ExitStack() as ctx:
            pool = lambda name, bufs, **kw: ctx.enter_context(
                tc.tile_pool(name=name, bufs=bufs, **kw))
            xt_pool = pool("xt", CT)
            wqk_pool = pool("wqkp", 3)
            w768_pool = pool("w768", 2 * CT)      # wv + wpt
            vaug_pool = pool("vaug", KT)
            st_pool = pool("stp", 6)
            ktq_pool = pool("ktq", 8)
            et_pool = pool("etp", 16)
            avs_pool = pool("avsp", 3)
            iv_pool = pool("ivp", 4)
            bc_pool = pool("bcp", 3)
            ost_pool = pool("ostp", 3)
            out_pool = pool("outp", CT)
            y_pool = pool("yp", 3)
            cst_pool = pool("cst", 1)
            sc_ps = pool("sc_ps", 3, space="PSUM")    # [128,1024] supertiles
            av_ps = pool("av_ps", 1, space="PSUM")
            qk_ps = pool("qk_ps", 1, space="PSUM")

            # ---- PE warm-up: bridge the DMA lead-in so the HAM clock gate
            # opens before real work arrives ---------------------------------
            wur = cst_pool.tile([128, 512], BF16, tag="wur")
            nc.vector.memset(wur, 0.0)
            for i in range(10):
                wps = qk_ps.tile([128, 512], F32, tag="qk", name=f"wu{i}")
                nc.tensor.matmul(wps, wur[:, 0:128], wur,
                                 start=True, stop=True)

            # ---- loads -----------------------------------------------------
            def load_wqk(h):
                wt = wqk_pool.tile([128, C], BF16, tag="wqk", name=f"wqk{h}")
                nc.sync.dma_start(out=wt, in_=wqk[h, :, :])
                return wt

            wts0 = load_wqk(0)

            xt = []
            for c in range(CT):
                t = xt_pool.tile([128, N], BF16, tag="xt", name=f"xt{c}")
                nc.sync.dma_start(out=t, in_=xT[c * 128:(c + 1) * 128, :])
                xt.append(t)

            bias_bc = cst_pool.tile([128, C], F32, tag="biasbc")
            nc.sync.dma_start(out=bias_bc, in_=bias[:, :].to_broadcast([128, C]))
            ones12 = cst_pool.tile([128, H], BF16, tag="ones12")
            nc.vector.memset(ones12, 1.0)

            wvt = []
            for c in range(CT):
                t = w768_pool.tile([128, C], BF16, tag="w768", name=f"wv{c}")
                nc.sync.dma_start(out=t, in_=wv[c, :, :])
                wvt.append(t)

            # ---- per-head q/k projection -----------------------------------
            def qk_project(h, wt):
                """q (rows 0-63) and k (rows 64-127), transposed layout."""
                sts, kts, qds = [], [], []
                for qc in range(QC):
                    pqk = qk_ps.tile([128, 512], F32, tag="qk",
                                     name=f"pqk{h}_{qc}")
                    for c in range(CT):
                        nc.tensor.matmul(
                            pqk, wt[:, c * 128:(c + 1) * 128],
                            xt[c][:, qc * 512:(qc + 1) * 512],
                            start=(c == 0), stop=(c == CT - 1),
                        )
                    st = st_pool.tile([128, 512], BF16, tag="st",
                                      name=f"st{h}_{qc}")
                    nc.vector.tensor_copy(st, pqk)
                    # k rows of EVEN key-chunks also needed at partitions 0-63
                    # (row-tile 0); q rows duplicated at partitions 64-127 for
                    # the odd-chunk matmuls on row-tile 1.
                    kt_t = ktq_pool.tile([128, 512], BF16, tag="ktq",
                                         name=f"kt{h}_{qc}")
                    nc.sync.dma_start(out=kt_t[0:64, :], in_=st[64:128, :])
                    qd = ktq_pool.tile([128, 512], BF16, tag="ktq",
                                       name=f"qd{h}_{qc}")
                    nc.sync.dma_start(out=qd[64:128, :], in_=st[0:64, :])
                    sts.append(st)
                    kts.append(kt_t)
                    qds.append(qd)
                return sts, kts, qds

            # ---- v_aug[tt] builders ---------------------------------------
            # all 8 tiles live for the whole kernel; write the ones columns
            # up-front so the hot loop's DVE queue stays short
            vaug = [vaug_pool.tile([128, VW], BF16, tag="vaug",
                                   name=f"vaug{tt}") for tt in range(KT)]
            for tt in range(KT):
                ones_ap = bass.AP(tensor=vaug[tt].tensor,
                                  offset=vaug[tt].offset + 64,
                                  ap=[vaug[tt].ap[0], [65, H]])
                nc.vector.tensor_copy(ones_ap, ones12)

            def build_vaug(tt):
                pv = sc_ps.tile([128, 1024], F32, tag="sc", name=f"pv{tt}")
                for c in range(CT):
                    xs = xt[c][:, tt * 128:(tt + 1) * 128]
                    nc.tensor.matmul(pv[:, 0:512], xs, wvt[c][:, 0:512],
                                     start=(c == 0), stop=(c == CT - 1))
                    nc.tensor.matmul(pv[:, 512:768], xs, wvt[c][:, 512:768],
                                     start=(c == 0), stop=(c == CT - 1))
                va = vaug[tt]
                dst = bass.AP(tensor=va.tensor, offset=va.offset,
                              ap=[va.ap[0], [65, H], [1, 64]])
                nc.vector.tensor_copy(dst, pv[:, 0:768])

            # ---- output accumulator tiles (c-major, [128, N]) --------------
            outT = [out_pool.tile([128, N], BF16, tag="outT", name=f"outT{i}")
                    for i in range(CT)]

            def emit_sc(slot, qc, kt, sk):
                sts, kts, qds = sk
                cs = slice((kt % 4) * 128, (kt % 4 + 1) * 128)
                if kt % 2 == 0:
                    nc.tensor.matmul(slot, kts[kt // 4][0:64, cs],
                                     sts[qc][0:64, :], start=True, stop=True,
                                     tile_position=(0, 0))
                else:
                    nc.tensor.matmul(slot, sts[kt // 4][64:128, cs],
                                     qds[qc][64:128, :], start=True, stop=True,
                                     tile_position=(64, 0))

            def emit_sc_chunk(h, qc, sk):
                """Scores + exp for one (head, query-chunk): 4 row-tiled
                pairs into [128,1024] supertiles, one Exp each."""
                ets = []
                for g in range(4):
                    ps = sc_ps.tile([128, 1024], F32, tag="sc",
                                    name=f"sc{h}_{qc}_{g}")
                    emit_sc(ps[:, 0:512], qc, 2 * g, sk)
                    emit_sc(ps[:, 512:1024], qc, 2 * g + 1, sk)
                    et = et_pool.tile([128, 1024], BF16, tag="et",
                                      name=f"et{h}_{qc}_{g}")
                    nc.scalar.activation(out=et, in_=ps, func=EXP)
                    ets.append(et)
                return ets

            def emit_av_chunk(h, qc, ets):
                """attn*v accumulation + softmax normalization for a chunk."""
                av = av_ps.tile([128, 512], F32, tag="av", name=f"av{h}_{qc}")
                for kt in range(KT):
                    nc.tensor.matmul(av, vaug[kt][:, h * 65:h * 65 + 128],
                                     ets[kt // 2][:, (kt % 2) * 512:
                                                  (kt % 2 + 1) * 512],
                                     start=(kt == 0), stop=(kt == KT - 1))
                avs = avs_pool.tile([65, 512], F32, tag="avs",
                                    name=f"avs{h}_{qc}")
                nc.vector.tensor_copy(avs, av[0:65, :])
                # row 64 of avs = softmax denominators for this q chunk.
                sm0 = iv_pool.tile([1, 512], F32, tag="sm0",
                                   name=f"sm0{h}_{qc}")
                nc.sync.dma_start(out=sm0, in_=avs[64:65, :])
                iv0 = iv_pool.tile([1, 512], F32, tag="iv0",
                                   name=f"iv0{h}_{qc}")
                nc.vector.reciprocal_approx_fast(out=iv0, in_=sm0)
                bc = bc_pool.tile([64, 512], F32, tag="bc", name=f"bc{h}_{qc}")
                nc.gpsimd.partition_broadcast(bc, iv0)
                ct_i = h // 2
                if h % 2 == 0:
                    nc.vector.tensor_mul(
                        outT[ct_i][0:64, qc * 512:(qc + 1) * 512],
                        avs[0:64, :], bc)
                else:
                    ost = ost_pool.tile([64, 512], BF16, tag="ost",
                                        name=f"ost{h}_{qc}")
                    nc.vector.tensor_mul(ost, avs[0:64, :], bc)
                    nc.sync.dma_start(
                        out=outT[ct_i][64:128, qc * 512:(qc + 1) * 512],
                        in_=ost)

            wptt = None

            def emit_proj(tts):
                for tt in tts:
                    py = sc_ps.tile([128, 1024], F32, tag="sc",
                                    name=f"py{tt}")
                    for c in range(CT):
                        os_ = outT[c][:, tt * 128:(tt + 1) * 128]
                        nc.tensor.matmul(py[:, 0:512], os_, wptt[c][:, 0:512],
                                         start=(c == 0), stop=(c == CT - 1))
                        nc.tensor.matmul(py[:, 512:768], os_,
                                         wptt[c][:, 512:768],
                                         start=(c == 0), stop=(c == CT - 1))
                    ysb = y_pool.tile([128, C], F32, tag="y", name=f"y{tt}")
                    nc.vector.tensor_add(ysb, py[:, 0:768], bias_bc)
                    nc.sync.dma_start(out=y[tt * 128:(tt + 1) * 128, :],
                                      in_=ysb)

            # ---- software-pipelined schedule -------------------------------
            head_order = list(range(H))
            head_order[10], head_order[11] = head_order[11], head_order[10]
            chunks = [(h, qc) for h in head_order for qc in range(QC)]
            pv_slots = {0: (0, 1), 1: (2, 3), 2: (4, 5), 3: (6, 7)}

            sk_by_head = {head_order[0]: qk_project(head_order[0], wts0)}
            ets_store = {}
            av_done = 0

            for s, (h, qc) in enumerate(chunks):
                ets_store[s] = emit_sc_chunk(h, qc, sk_by_head[h])
                if s in pv_slots:
                    for tt in pv_slots[s]:
                        build_vaug(tt)
                # av chunks before qk: their DVE drain chain must precede the
                # qk casts in the FIFO (it releases the av PSUM bank)
                target = 0 if s < 3 else (s - 2 if s < 8 else s)
                while av_done < target:
                    ch, cq = chunks[av_done]
                    emit_av_chunk(ch, cq, ets_store.pop(av_done))
                    av_done += 1
                    if av_done == len(chunks) - 1:
                        emit_proj(range(0, 4))
                if qc == 0 and 2 * (s // 2) + 2 < len(chunks):
                    hn = chunks[s + 2][0]
                    sk_by_head[hn] = qk_project(hn, load_wqk(hn))
                if (h, qc) == (5, 1):
                    wptt = []
                    for c in range(CT):
                        t = w768_pool.tile([128, C], BF16, tag="w768",
                                           name=f"wpt{c}")
                        nc.sync.dma_start(out=t, in_=wpt[c, :, :])
                        wptt.append(t)

            while av_done < len(chunks):
                ch, cq = chunks[av_done]
                emit_av_chunk(ch, cq, ets_store.pop(av_done))
                av_done += 1
                if av_done == len(chunks) - 1:
                    emit_proj(range(0, 4))
            emit_proj(range(4, KT))

    nc.finalize()
    return nc


def _get_nc():
    global _NC_CACHE
    if _NC_CACHE is None:
        _NC_CACHE = _build()
    return _NC_CACHE


def _host_prep(x, Wqkv, Wproj, bproj, Aq, Bq, Av, Bv):
    """Fold LoRA + score scale into the weights; lay out and cast to bf16."""
    W = Wqkv.astype(np.float64)
    Wq = W[0:C].reshape(H, HD, C)
    Wk = W[C:2 * C].reshape(H, HD, C)
    Wv_ = W[2 * C:3 * C].reshape(H, HD, C)
    ABq = Aq.astype(np.float64) @ Bq.astype(np.float64)   # [HD, HD]
    ABv = Av.astype(np.float64) @ Bv.astype(np.float64)
    Wq = Wq + np.einsum('ed,hec->hdc', ABq, Wq)           # (I+AB).T @ Wq per head
    Wv_ = Wv_ + np.einsum('ed,hec->hdc', ABv, Wv_)
    Wq = Wq * SCALE                                       # fold softmax scale

    # wqk[h] = [K=c-rows(128), 6 c-tiles of (q_h cols(64) ++ k_h cols(64))]
    wqk = np.empty((H, 128, C), np.float32)
    for h in range(H):
        for c in range(CT):
            cs = slice(c * 128, (c + 1) * 128)
            wqk[h, :, c * 128:c * 128 + 64] = Wq[h][:, cs].T
            wqk[h, :, c * 128 + 64:(c + 1) * 128] = Wk[h][:, cs].T

    # wv[c] = [K=c-rows(128), all 768 v output features]
    WvT = Wv_.reshape(C, C).T.astype(np.float32)          # [c_in, v_out]
    wv = np.ascontiguousarray(WvT.reshape(CT, 128, C))

    # wpt[c] = Wproj.T c-tiles: [K=c(128), e(768)]
    WpT = Wproj.astype(np.float32).T                      # [c, e]
    wpt = np.ascontiguousarray(WpT.reshape(CT, 128, C))

    bf = ml_dtypes.bfloat16
    wqk = wqk.astype(bf)
    wv = wv.astype(bf)
    wpt = wpt.astype(bf)
    bias = bproj.astype(np.float32).reshape(1, C)

    per_core = []
    for b in range(B):
        xTb = np.ascontiguousarray(x[b].astype(np.float32).T.astype(bf))
        per_core.append({"xT": xTb, "wqk": wqk, "wv": wv, "wpt": wpt,
                         "bias": bias})
    return per_core


def kernel(x, Wqkv, Wproj, bproj, Aq, Bq, Av, Bv, _trace=False):
    x = np.asarray(x)
    in_maps = _host_prep(np.asarray(x), np.asarray(Wqkv), np.asarray(Wproj),
                         np.asarray(bproj), np.asarray(Aq), np.asarray(Bq),
                         np.asarray(Av), np.asarray(Bv))
    nc = _get_nc()
    res = run_bass_kernel_spmd(nc, in_maps, core_ids=list(range(N_CORES)),
                               trace=_trace)
    out = np.stack([res.results[b]["y"] for b in range(B)], axis=0)
    if _trace:
        kernel._last_result = res
    return out.astype(np.float32)
